# revision 8
# baseline (speedup 1.0000x reference)
"""Trainium2 Bass kernel for nn_DiffuRNNLayer (B=8, N=2048, D=1024).

Sharding: data-parallel over batch — one batch element per NeuronCore (8 cores).
Per-core kernel works in "layout B" ([d on partitions, n on free]) with the
input pre-transposed on the host.  Four phases per core:
  A: Q/K/V projections (+elu+1), K_sum accumulation; spill Qp (layout B) and
     Kp/V (layout A) to HBM scratch.
  B: KV = Kp^T V accumulation over all tokens (PSUM-resident, 2 e-halves).
  C: acc = dwconv''(x) + MLP(x) + tokenmixer(LN(x))  (diffusion residual and
     all constant per-channel biases folded into the dwconv'' taps host-side);
     spill acc.
  D: attn numerator with 1/norm folded into Qp, acc += attn; LN1; FFN
     residual; LN2; write y^T.
Host transposes x/weights in, and the output back out.
"""

import numpy as np
import ml_dtypes
from contextlib import ExitStack

import concourse.bass as bass
import concourse.bacc as bacc
import concourse.tile as tile
import concourse.mybir as mybir
from concourse.bass_utils import run_bass_kernel_spmd

F32 = mybir.dt.float32
BF16 = mybir.dt.bfloat16
AF = mybir.ActivationFunctionType
OP = mybir.AluOpType
BF16_NP = ml_dtypes.bfloat16

P = 128
D = 1024
DO = D // P  # 8 chunks of the channel dim

# pp param-plane indices (per-partition params, laid out [128, DO, NP])
(C0, C1, C2, CB, T0, T1, T2, TCB1, U0, U1, U2,
 TMG, TMB, N1G, N1B, N2G, N2B, LUB1, FFB1, FFB2) = range(20)
NPARAM = 20


def _ceil_div(a, b):
    return (a + b - 1) // b


def build_nc(N=2048, NT=512, use_bq=False, use_bk=False, use_bv=False,
             use_tmb=False, use_n1b=False, use_n2b=False, debug=False):
    NTILES = N // NT
    NCH = NT // P          # 128-token chunks per tile
    TOTCH = N // P
    W = NT + 4             # phase-C tile width with +-2 halo
    assert N % NT == 0 and NT % P == 0

    nc = bacc.Bacc(None, target_bir_lowering=False, debug=debug)

    xT_d = nc.dram_tensor("x_T", [D, N], BF16, kind="ExternalInput")
    w_d = {}
    for name in ("wqT", "wkT", "wvT", "w1T", "w2T", "f1T", "f2T"):
        w_d[name] = nc.dram_tensor(name, [D, D], BF16, kind="ExternalInput")
    pp_d = nc.dram_tensor("pp", [P, DO, NPARAM], F32, kind="ExternalInput")
    rows_d = nc.dram_tensor("rows", [1, 3 * D], BF16, kind="ExternalInput")
    yT_d = nc.dram_tensor("y_T", [D, N], F32, kind="ExternalOutput")

    qp_sp = nc.dram_tensor("qp_sp", [D, N], BF16)
    kp_sp = nc.dram_tensor("kp_sp", [N, D], BF16)
    v_sp = nc.dram_tensor("v_sp", [N, D], BF16)
    acc_sp = nc.dram_tensor("acc_sp", [D, N], F32)

    xT = xT_d.rearrange("(o p) n -> p o n", p=P)
    wr = {k: v.rearrange("(o p) n -> p o n", p=P) for k, v in w_d.items()}
    qp_r = qp_sp.rearrange("(o p) n -> p o n", p=P)
    kp_r = kp_sp.rearrange("(c p) d -> p c d", p=P)
    v_r = v_sp.rearrange("(c p) d -> p c d", p=P)
    acc_r = acc_sp.rearrange("(o p) n -> p o n", p=P)
    yT = yT_d.rearrange("(o p) n -> p o n", p=P)

    with tile.TileContext(nc) as tc, ExitStack() as top:
        persist = top.enter_context(tc.tile_pool(name="persist", bufs=1))
        pp = persist.tile([P, DO, NPARAM], F32)
        nc.sync.dma_start(pp, pp_d[:])
        rows = persist.tile([1, 3 * D], BF16)
        nc.sync.dma_start(rows, rows_d[:])
        ones_row = persist.tile([1, NT], BF16)
        nc.vector.memset(ones_row, 1.0)
        ones_1p_bf = persist.tile([1, P], BF16)
        nc.vector.memset(ones_1p_bf, 1.0)
        ones_1p_f32 = persist.tile([1, P], F32)
        nc.vector.memset(ones_1p_f32, 1.0)
        ones_col = persist.tile([P, 1], BF16)
        nc.vector.memset(ones_col, 1.0)
        ones_one = persist.tile([1, 1], BF16)
        nc.vector.memset(ones_one, 1.0)
        ksrow_sb = persist.tile([1, D], BF16)
        onesD_bf = persist.tile([P, P], BF16)
        nc.vector.memset(onesD_bf, 1.0 / D)
        onesD_f32 = persist.tile([P, P], F32)
        nc.vector.memset(onesD_f32, 1.0 / D)
        eps_ln = persist.tile([P, 1], F32)
        nc.vector.memset(eps_ln, 1e-5)
        kv_sb = persist.tile([P, DO, D], BF16)
        ksum_sb = persist.tile([P, DO, 1], BF16)

        def stats_mm(psum, lhs_ones, rhs3, width):
            """Accumulate over DO k-chunks: psum[:, j] = mean over channel dim,
            replicated across partitions.  rhs3: [P, DO, width]."""
            for c0 in range(0, width, 512):
                cw = min(512, width - c0)
                for kc in range(DO):
                    nc.tensor.matmul(psum[:, c0:c0 + cw], lhs_ones,
                                     rhs3[:, kc, c0:c0 + cw],
                                     start=(kc == 0), stop=(kc == DO - 1))

        # ---------------- Phase A: QKV ----------------
        with ExitStack() as ph:
            wpool = ph.enter_context(tc.tile_pool(name="wA", bufs=1))
            wq_sb = wpool.tile([P, DO, D], BF16, tag="wq")
            nc.sync.dma_start(wq_sb, wr["wqT"])
            wk_sb = wpool.tile([P, DO, D], BF16, tag="wk")
            nc.sync.dma_start(wk_sb, wr["wkT"])
            wv_sb = wpool.tile([P, DO, D], BF16, tag="wv")
            nc.sync.dma_start(wv_sb, wr["wvT"])
            io = ph.enter_context(tc.tile_pool(name="ioA", bufs=2))
            ev = ph.enter_context(tc.tile_pool(name="evA", bufs=3))
            ps = ph.enter_context(tc.tile_pool(name="psA", bufs=2, space="PSUM"))
            ksp = ph.enter_context(tc.tile_pool(name="ksA", bufs=1, space="PSUM"))
            ps_ks = ksp.tile([1, D], F32, tag="ksrow")

            for it in range(NTILES):
                n0 = it * NT
                x_t = io.tile([P, DO, NT], BF16, tag="xA")
                nc.sync.dma_start(x_t, xT[:, :, n0:n0 + NT])

                # ---- Q: layout B, out [dout-chunk, n] ----
                qp_t = io.tile([P, DO, NT], BF16, tag="qpA")
                for dc in range(DO):
                    ps_q = ps.tile([P, NT], F32, tag="psq")
                    for kc in range(DO):
                        nc.tensor.matmul(ps_q, wq_sb[:, kc, dc * P:(dc + 1) * P],
                                         x_t[:, kc, :], start=(kc == 0),
                                         stop=(kc == DO - 1 and not use_bq))
                    if use_bq:
                        nc.tensor.matmul(ps_q, rows[0:1, dc * P:(dc + 1) * P],
                                         ones_row[0:1, :], start=False, stop=True)
                    m_t = ev.tile([P, NT], BF16, tag="mA")
                    nc.vector.tensor_scalar_min(m_t, ps_q, 0.0)
                    e_t = ev.tile([P, NT], BF16, tag="eA")
                    nc.scalar.activation(e_t, m_t, AF.Exp)
                    # elu(q)+1 = max(q,0) + exp(min(q,0))
                    nc.vector.scalar_tensor_tensor(qp_t[:, dc, :], ps_q, 0.0, e_t,
                                                   OP.max, OP.add)
                nc.sync.dma_start(qp_r[:, :, n0:n0 + NT], qp_t)

                # ---- K, V: layout A, out [token-chunk, dout] ----
                kp_t = io.tile([P, NCH, D], BF16, tag="kpA")
                v_t = io.tile([P, NCH, D], BF16, tag="vA")
                for ch in range(NCH):
                    cs = slice(ch * P, (ch + 1) * P)
                    for half in range(D // 512):
                        hs = slice(half * 512, (half + 1) * 512)
                        ps_k = ps.tile([P, 512], F32, tag="pskv")
                        for kc in range(DO):
                            nc.tensor.matmul(ps_k, x_t[:, kc, cs], wk_sb[:, kc, hs],
                                             start=(kc == 0),
                                             stop=(kc == DO - 1 and not use_bk))
                        if use_bk:
                            nc.tensor.matmul(ps_k, ones_1p_bf[0:1, :],
                                             rows[0:1, D + half * 512:D + (half + 1) * 512],
                                             start=False, stop=True)
                        m2 = ev.tile([P, 512], BF16, tag="mA2")
                        nc.vector.tensor_scalar_min(m2, ps_k, 0.0)
                        e2 = ev.tile([P, 512], BF16, tag="eA2")
                        nc.scalar.activation(e2, m2, AF.Exp)
                        nc.vector.scalar_tensor_tensor(kp_t[:, ch, hs], ps_k, 0.0,
                                                       e2, OP.max, OP.add)

                        ps_v = ps.tile([P, 512], F32, tag="pskv")
                        for kc in range(DO):
                            nc.tensor.matmul(ps_v, x_t[:, kc, cs], wv_sb[:, kc, hs],
                                             start=(kc == 0),
                                             stop=(kc == DO - 1 and not use_bv))
                        if use_bv:
                            nc.tensor.matmul(ps_v, ones_1p_bf[0:1, :],
                                             rows[0:1, 2 * D + half * 512:2 * D + (half + 1) * 512],
                                             start=False, stop=True)
                        nc.scalar.activation(v_t[:, ch, hs], ps_v, AF.Copy)
                    # K_sum accumulation as a [1, D] row (tokens on partitions)
                    for half in range(D // 512):
                        hs = slice(half * 512, (half + 1) * 512)
                        nc.tensor.matmul(ps_ks[0:1, hs], ones_col, kp_t[:, ch, hs],
                                         start=(it == 0 and ch == 0),
                                         stop=(it == NTILES - 1 and ch == NCH - 1))
                nc.sync.dma_start(kp_r[:, it * NCH:(it + 1) * NCH, :], kp_t)
                nc.sync.dma_start(v_r[:, it * NCH:(it + 1) * NCH, :], v_t)
            nc.scalar.activation(ksrow_sb, ps_ks[0:1, :], AF.Copy)
            # transpose K_sum row -> per-partition column layout [P, DO]
            ps_ksc = ksp.tile([P, DO], F32, tag="kscol")
            for dc in range(DO):
                nc.tensor.matmul(ps_ksc[:, dc:dc + 1],
                                 ksrow_sb[0:1, dc * P:(dc + 1) * P],
                                 ones_one[0:1, 0:1], start=True, stop=True)
            nc.scalar.activation(ksum_sb[:, :, 0], ps_ksc, AF.Copy)

        # ---------------- Phase B: KV accumulation ----------------
        with ExitStack() as ph:
            io = ph.enter_context(tc.tile_pool(name="ioB", bufs=4))
            ps = ph.enter_context(tc.tile_pool(name="psB", bufs=1, space="PSUM"))
            for eh in range(D // 512):
                hs = slice(eh * 512, (eh + 1) * 512)
                kv_ps = [ps.tile([P, 512], F32, tag=f"kvps{d}", name=f"kvps{d}")
                         for d in range(DO)]
                for ch in range(TOTCH):
                    kp_c = io.tile([P, D], BF16, tag="kpB")
                    nc.sync.dma_start(kp_c, kp_r[:, ch, :])
                    v_c = io.tile([P, 512], BF16, tag="vB")
                    nc.sync.dma_start(v_c, v_r[:, ch, hs])
                    for dc in range(DO):
                        nc.tensor.matmul(kv_ps[dc], kp_c[:, dc * P:(dc + 1) * P],
                                         v_c, start=(ch == 0), stop=(ch == TOTCH - 1))
                for dc in range(DO):
                    nc.scalar.activation(kv_sb[:, dc, hs], kv_ps[dc], AF.Copy)

        # ---------------- Phase C: conv'' + local MLP + token mixer ----------------
        with ExitStack() as ph:
            wpool = ph.enter_context(tc.tile_pool(name="wC", bufs=1))
            w1_sb = wpool.tile([P, DO, D], BF16, tag="w1")
            nc.sync.dma_start(w1_sb, wr["w1T"])
            w2_sb = wpool.tile([P, DO, D], BF16, tag="w2")
            nc.sync.dma_start(w2_sb, wr["w2T"])
            io = ph.enter_context(tc.tile_pool(name="ioC", bufs=2))
            mid = ph.enter_context(tc.tile_pool(name="midC", bufs=1))
            sm = ph.enter_context(tc.tile_pool(name="smC", bufs=2))
            ps = ph.enter_context(tc.tile_pool(name="psC", bufs=2, space="PSUM"))
            pst = ph.enter_context(tc.tile_pool(name="pstC", bufs=1, space="PSUM"))

            for it in range(NTILES):
                n0 = it * NT
                x_t = io.tile([P, DO, W], BF16, tag="xC")
                lo, hi = n0 - 2, n0 + NT + 2
                if lo < 0:
                    nc.vector.memset(x_t[:, :, 0:2], 0.0)
                    nc.sync.dma_start(x_t[:, :, 2:W], xT[:, :, 0:hi])
                elif hi > N:
                    nc.vector.memset(x_t[:, :, W - 2:W], 0.0)
                    nc.sync.dma_start(x_t[:, :, 0:W - 2], xT[:, :, lo:N])
                else:
                    nc.sync.dma_start(x_t, xT[:, :, lo:hi])

                acc = io.tile([P, DO, NT], F32, tag="accC")
                # diffusion dwconv'' (residual + all constant biases folded in)
                for o in range(DO):
                    nc.vector.tensor_scalar(acc[:, o, :], x_t[:, o, 2:NT + 2],
                                            pp[:, o, C1:C1 + 1], pp[:, o, CB:CB + 1],
                                            OP.mult, OP.add)
                for o in range(DO):
                    nc.vector.scalar_tensor_tensor(acc[:, o, :], x_t[:, o, 1:NT + 1],
                                                   pp[:, o, C0:C0 + 1], acc[:, o, :],
                                                   OP.mult, OP.add)
                for o in range(DO):
                    nc.vector.scalar_tensor_tensor(acc[:, o, :], x_t[:, o, 3:NT + 3],
                                                   pp[:, o, C2:C2 + 1], acc[:, o, :],
                                                   OP.mult, OP.add)

                # local MLP
                h1_t = mid.tile([P, DO, NT], BF16, tag="h1")
                for dc in range(DO):
                    ps_h = ps.tile([P, NT], F32, tag="psh1")
                    for kc in range(DO):
                        nc.tensor.matmul(ps_h, w1_sb[:, kc, dc * P:(dc + 1) * P],
                                         x_t[:, kc, 2:NT + 2],
                                         start=(kc == 0), stop=(kc == DO - 1))
                    nc.scalar.activation(h1_t[:, dc, :], ps_h, AF.Gelu,
                                         bias=pp[:, dc, LUB1:LUB1 + 1])
                for dc in range(DO):
                    ps_h = ps.tile([P, NT], F32, tag="psh2")
                    for kc in range(DO):
                        nc.tensor.matmul(ps_h, w2_sb[:, kc, dc * P:(dc + 1) * P],
                                         h1_t[:, kc, :],
                                         start=(kc == 0), stop=(kc == DO - 1))
                    nc.vector.tensor_add(acc[:, dc, :], acc[:, dc, :], ps_h)

                # token mixer: LN over channels (stats via ones-matmul)
                sq_t = mid.tile([P, DO, W], BF16, tag="sqC")
                nc.scalar.activation(sq_t, x_t, AF.Square)
                ps_m = pst.tile([P, W], F32, tag="psm")
                stats_mm(ps_m, onesD_bf, x_t, W)
                ps_s = pst.tile([P, W], F32, tag="pss")
                stats_mm(ps_s, onesD_bf, sq_t, W)
                var = sm.tile([P, W], F32, tag="var")
                nc.scalar.activation(var, ps_m, AF.Square)
                nc.vector.tensor_sub(var, ps_s, var)
                std = sm.tile([P, W], F32, tag="std")
                nc.scalar.activation(std, var, AF.Sqrt, bias=eps_ln[:, 0:1])
                rstd = sm.tile([P, W], F32, tag="rstd")
                nc.vector.reciprocal(rstd, std)
                u_t = mid.tile([P, DO, W], BF16, tag="uC")
                for o in range(DO):
                    nc.vector.tensor_sub(u_t[:, o, :], x_t[:, o, :], ps_m)
                xm_t = mid.tile([P, DO, W], BF16, tag="xm")
                for o in range(DO):
                    nc.vector.scalar_tensor_tensor(xm_t[:, o, :], u_t[:, o, :],
                                                   pp[:, o, TMG:TMG + 1], rstd,
                                                   OP.mult, OP.mult)
                if use_tmb:
                    for o in range(DO):
                        nc.vector.tensor_scalar_add(xm_t[:, o, :], xm_t[:, o, :],
                                                    pp[:, o, TMB:TMB + 1])
                # conv1 valid on [1, W-1)
                t_t = mid.tile([P, DO, W], BF16, tag="t1")
                for o in range(DO):
                    nc.vector.tensor_scalar(t_t[:, o, 1:W - 1], xm_t[:, o, 1:W - 1],
                                            pp[:, o, T1:T1 + 1], pp[:, o, TCB1:TCB1 + 1],
                                            OP.mult, OP.add)
                for o in range(DO):
                    nc.vector.scalar_tensor_tensor(t_t[:, o, 1:W - 1],
                                                   xm_t[:, o, 0:W - 2],
                                                   pp[:, o, T0:T0 + 1],
                                                   t_t[:, o, 1:W - 1], OP.mult, OP.add)
                for o in range(DO):
                    nc.vector.scalar_tensor_tensor(t_t[:, o, 1:W - 1],
                                                   xm_t[:, o, 2:W],
                                                   pp[:, o, T2:T2 + 1],
                                                   t_t[:, o, 1:W - 1], OP.mult, OP.add)
                t2_t = mid.tile([P, DO, W], BF16, tag="t2")
                nc.scalar.activation(t2_t[:, :, 1:W - 1], t_t[:, :, 1:W - 1], AF.Gelu)
                # conv2's zero padding applies to gelu(conv1) outside the sequence
                if it == 0:
                    nc.vector.memset(t2_t[:, :, 1:2], 0.0)
                if it == NTILES - 1:
                    nc.vector.memset(t2_t[:, :, W - 2:W - 1], 0.0)
                # conv2 valid on [2, W-2) == the NT output cols
                t3_t = mid.tile([P, DO, NT], BF16, tag="t3")
                for o in range(DO):
                    nc.vector.tensor_scalar(t3_t[:, o, :], t2_t[:, o, 2:W - 2],
                                            pp[:, o, U1:U1 + 1], None, OP.mult)
                for o in range(DO):
                    nc.vector.scalar_tensor_tensor(t3_t[:, o, :], t2_t[:, o, 1:W - 3],
                                                   pp[:, o, U0:U0 + 1], t3_t[:, o, :],
                                                   OP.mult, OP.add)
                for o in range(DO):
                    nc.vector.scalar_tensor_tensor(t3_t[:, o, :], t2_t[:, o, 3:W - 1],
                                                   pp[:, o, U2:U2 + 1], t3_t[:, o, :],
                                                   OP.mult, OP.add)
                for o in range(DO):
                    nc.vector.tensor_add(acc[:, o, :], acc[:, o, :], t3_t[:, o, :])
                nc.sync.dma_start(acc_r[:, :, n0:n0 + NT], acc)

        # ---------------- Phase D: attention + LN1 + FFN + LN2 ----------------
        with ExitStack() as ph:
            wpool = ph.enter_context(tc.tile_pool(name="wD", bufs=1))
            f1_sb = wpool.tile([P, DO, D], BF16, tag="f1")
            nc.sync.dma_start(f1_sb, wr["f1T"])
            f2_sb = wpool.tile([P, DO, D], BF16, tag="f2")
            nc.sync.dma_start(f2_sb, wr["f2T"])
            io = ph.enter_context(tc.tile_pool(name="ioD", bufs=2))
            mid = ph.enter_context(tc.tile_pool(name="midD", bufs=1))
            sm = ph.enter_context(tc.tile_pool(name="smD", bufs=1))
            ps = ph.enter_context(tc.tile_pool(name="psD", bufs=2, space="PSUM"))
            pst = ph.enter_context(tc.tile_pool(name="pstD", bufs=1, space="PSUM"))

            for it in range(NTILES):
                n0 = it * NT
                qp_t = io.tile([P, DO, NT], BF16, tag="qpD")
                nc.sync.dma_start(qp_t, qp_r[:, :, n0:n0 + NT])
                acc_t = io.tile([P, DO, NT], F32, tag="accD")
                nc.sync.dma_start(acc_t, acc_r[:, :, n0:n0 + NT])

                # norm row = Qp . K_sum  (contract channels), then 1/(norm+eps)
                ps_n = pst.tile([P, NT], F32, tag="psn")
                for kc in range(DO):
                    nc.tensor.matmul(ps_n[0:1, :], ksum_sb[:, kc, :], qp_t[:, kc, :],
                                     start=(kc == 0), stop=(kc == DO - 1))
                nr = sm.tile([1, NT], F32, tag="nr")
                nc.vector.tensor_scalar_add(nr, ps_n[0:1, :], 1e-6)
                rr = sm.tile([1, NT], F32, tag="rr")
                nc.vector.reciprocal(rr, nr)
                ps_rep = pst.tile([P, NT], F32, tag="psrep")
                nc.tensor.matmul(ps_rep, ones_1p_f32[0:1, :], rr, start=True, stop=True)
                for kc in range(DO):
                    nc.vector.tensor_mul(qp_t[:, kc, :], qp_t[:, kc, :], ps_rep)

                # numerator (with 1/norm folded into Qp'), accumulate into acc
                for ec in range(DO):
                    ps_u = ps.tile([P, NT], F32, tag="psnum")
                    for kc in range(DO):
                        nc.tensor.matmul(ps_u, kv_sb[:, kc, ec * P:(ec + 1) * P],
                                         qp_t[:, kc, :],
                                         start=(kc == 0), stop=(kc == DO - 1))
                    nc.vector.tensor_add(acc_t[:, ec, :], acc_t[:, ec, :], ps_u)

                # LN1
                sq_t = mid.tile([P, DO, NT], BF16, tag="sqD")
                nc.scalar.activation(sq_t, acc_t, AF.Square)
                ps_m1 = pst.tile([P, NT], F32, tag="psm1")
                stats_mm(ps_m1, onesD_f32, acc_t, NT)
                ps_s1 = pst.tile([P, NT], F32, tag="pss1")
                stats_mm(ps_s1, onesD_bf, sq_t, NT)
                var1 = sm.tile([P, NT], F32, tag="varD")
                nc.scalar.activation(var1, ps_m1, AF.Square)
                nc.vector.tensor_sub(var1, ps_s1, var1)
                std1 = sm.tile([P, NT], F32, tag="stdD")
                nc.scalar.activation(std1, var1, AF.Sqrt, bias=eps_ln[:, 0:1])
                rstd1 = sm.tile([P, NT], F32, tag="rstdD")
                nc.vector.reciprocal(rstd1, std1)
                u1_t = mid.tile([P, DO, NT], BF16, tag="u1")
                for o in range(DO):
                    nc.vector.tensor_sub(u1_t[:, o, :], acc_t[:, o, :], ps_m1)
                y1_t = mid.tile([P, DO, NT], BF16, tag="y1")
                for o in range(DO):
                    nc.vector.scalar_tensor_tensor(y1_t[:, o, :], u1_t[:, o, :],
                                                   pp[:, o, N1G:N1G + 1], rstd1,
                                                   OP.mult, OP.mult)
                if use_n1b:
                    for o in range(DO):
                        nc.vector.tensor_scalar_add(y1_t[:, o, :], y1_t[:, o, :],
                                                    pp[:, o, N1B:N1B + 1])

                # FFN
                f1h_t = mid.tile([P, DO, NT], BF16, tag="f1h")
                for dc in range(DO):
                    ps_f = ps.tile([P, NT], F32, tag="psf")
                    for kc in range(DO):
                        nc.tensor.matmul(ps_f, f1_sb[:, kc, dc * P:(dc + 1) * P],
                                         y1_t[:, kc, :],
                                         start=(kc == 0), stop=(kc == DO - 1))
                    nc.scalar.activation(f1h_t[:, dc, :], ps_f, AF.Gelu,
                                         bias=pp[:, dc, FFB1:FFB1 + 1])
                y2_t = mid.tile([P, DO, NT], F32, tag="y2")
                for dc in range(DO):
                    ps_f = ps.tile([P, NT], F32, tag="psf")
                    for kc in range(DO):
                        nc.tensor.matmul(ps_f, f2_sb[:, kc, dc * P:(dc + 1) * P],
                                         f1h_t[:, kc, :],
                                         start=(kc == 0), stop=(kc == DO - 1))
                    # y2 = (f2 + ff_b2) + y1
                    nc.vector.scalar_tensor_tensor(y2_t[:, dc, :], ps_f,
                                                   pp[:, dc, FFB2:FFB2 + 1],
                                                   y1_t[:, dc, :], OP.add, OP.add)

                # LN2 -> output
                sq2_t = mid.tile([P, DO, NT], BF16, tag="sqD")
                nc.scalar.activation(sq2_t, y2_t, AF.Square)
                ps_m2 = pst.tile([P, NT], F32, tag="psm1")
                stats_mm(ps_m2, onesD_f32, y2_t, NT)
                ps_s2 = pst.tile([P, NT], F32, tag="pss1")
                stats_mm(ps_s2, onesD_bf, sq2_t, NT)
                var2 = sm.tile([P, NT], F32, tag="varD")
                nc.scalar.activation(var2, ps_m2, AF.Square)
                nc.vector.tensor_sub(var2, ps_s2, var2)
                std2 = sm.tile([P, NT], F32, tag="stdD")
                nc.scalar.activation(std2, var2, AF.Sqrt, bias=eps_ln[:, 0:1])
                rstd2 = sm.tile([P, NT], F32, tag="rstdD")
                nc.vector.reciprocal(rstd2, std2)
                yo_t = mid.tile([P, DO, NT], F32, tag="yo")
                for o in range(DO):
                    nc.vector.tensor_sub(yo_t[:, o, :], y2_t[:, o, :], ps_m2)
                for o in range(DO):
                    nc.vector.scalar_tensor_tensor(yo_t[:, o, :], yo_t[:, o, :],
                                                   pp[:, o, N2G:N2G + 1], rstd2,
                                                   OP.mult, OP.mult)
                if use_n2b:
                    for o in range(DO):
                        nc.vector.tensor_scalar_add(yo_t[:, o, :], yo_t[:, o, :],
                                                    pp[:, o, N2B:N2B + 1])
                nc.sync.dma_start(yT[:, :, n0:n0 + NT], yo_t)

    nc.compile()
    return nc


def make_in_maps(inputs, n_cores=8):
    """Host-side preprocessing: fold constants, transpose, cast, shard."""
    x = np.asarray(inputs["x"], np.float32)
    B, N, D_ = x.shape
    dt = float(np.asarray(inputs["delta_t"]))

    def g(k):
        return np.asarray(inputs[k], np.float32)

    diff_w, diff_b = g("diff_w"), g("diff_b")
    tm_w1, tm_cb1 = g("tm_w1"), g("tm_cb1")
    tm_w2, tm_cb2 = g("tm_w2"), g("tm_cb2")

    pp = np.zeros((P, DO, NPARAM), np.float32)

    def put(i, v):
        pp[:, :, i] = v.reshape(DO, P).T

    put(C0, dt * diff_w[:, 0, 0])
    put(C1, dt * diff_w[:, 0, 1] + (1.0 - dt))
    put(C2, dt * diff_w[:, 0, 2])
    put(CB, dt * diff_b + g("lu_b2") + tm_cb2)
    put(T0, tm_w1[:, 0, 0])
    put(T1, tm_w1[:, 0, 1])
    put(T2, tm_w1[:, 0, 2])
    put(TCB1, tm_cb1)
    put(U0, tm_w2[:, 0, 0])
    put(U1, tm_w2[:, 0, 1])
    put(U2, tm_w2[:, 0, 2])
    put(TMG, g("tm_g"))
    put(TMB, g("tm_beta"))
    put(N1G, g("n1_g"))
    put(N1B, g("n1_b"))
    put(N2G, g("n2_g"))
    put(N2B, g("n2_b"))
    put(LUB1, g("lu_b1"))
    put(FFB1, g("ff_b1"))
    put(FFB2, g("ff_b2"))

    rows = np.zeros((1, 3 * D), np.float32)
    rows[0, 0:D] = g("bq")
    rows[0, D:2 * D] = g("bk")
    rows[0, 2 * D:3 * D] = g("bv")
    rows = rows.astype(BF16_NP)

    wt = {}
    for name, key in (("wqT", "wq"), ("wkT", "wk"), ("wvT", "wv"),
                      ("w1T", "lu_w1"), ("w2T", "lu_w2"),
                      ("f1T", "ff_w1"), ("f2T", "ff_w2")):
        wt[name] = np.ascontiguousarray(g(key).T).astype(BF16_NP)

    xT = np.ascontiguousarray(x.transpose(0, 2, 1)).astype(BF16_NP)

    flags = dict(
        use_bq=bool(np.any(g("bq"))),
        use_bk=bool(np.any(g("bk"))),
        use_bv=bool(np.any(g("bv"))),
        use_tmb=bool(np.any(g("tm_beta"))),
        use_n1b=bool(np.any(g("n1_b"))),
        use_n2b=bool(np.any(g("n2_b"))),
    )

    shared = {**wt, "pp": pp, "rows": rows}
    in_maps = [{**shared, "x_T": xT[b]} for b in range(n_cores)]
    return in_maps, flags, (B, N)


_NC_CACHE = {}


def kernel(**inputs):
    in_maps, flags, (B, N) = make_in_maps(inputs)
    key = (N, tuple(sorted(flags.items())))
    if key not in _NC_CACHE:
        _NC_CACHE[key] = build_nc(N=N, NT=512, **flags)
    nc = _NC_CACHE[key]
    res = run_bass_kernel_spmd(nc, in_maps, list(range(B)))
    y = np.stack([res.results[b]["y_T"] for b in range(B)])
    return np.ascontiguousarray(y.transpose(0, 2, 1)).astype(np.float32)


# revision 12
# speedup vs baseline: 1.0449x; 1.0449x over previous
"""Trainium2 Bass kernel for nn_DiffuRNNLayer (B=8, N=2048, D=1024).

Sharding: data-parallel over batch — one batch element per NeuronCore (8 cores).
Per-core kernel works in "layout B" ([d on partitions, n on free]) with the
input pre-transposed on the host.  Four phases per core:
  A: Q/K/V projections (+elu+1), K_sum accumulation; spill Qp (layout B) and
     Kp/V (layout A) to HBM scratch.
  B: KV = Kp^T V accumulation over all tokens (PSUM-resident, 2 e-halves).
  C: acc = dwconv''(x) + MLP(x) + tokenmixer(LN(x))  (diffusion residual and
     all constant per-channel biases folded into the dwconv'' taps host-side);
     spill acc.
  D: attn numerator with 1/norm folded into Qp, acc += attn; LN1; FFN
     residual; LN2; write y^T.
Host transposes x/weights in, and the output back out.
"""

import numpy as np
import ml_dtypes
from contextlib import ExitStack

import concourse.bass as bass
import concourse.bacc as bacc
import concourse.tile as tile
import concourse.mybir as mybir
from concourse.bass_utils import run_bass_kernel_spmd

F32 = mybir.dt.float32
BF16 = mybir.dt.bfloat16
AF = mybir.ActivationFunctionType
OP = mybir.AluOpType
BF16_NP = ml_dtypes.bfloat16

P = 128
D = 1024
DO = D // P  # 8 chunks of the channel dim

# pp param-plane indices (per-partition params, laid out [128, DO, NP])
(C0, C1, C2, CB, T0, T1, T2, TCB1, U0, U1, U2,
 TMG, TMB, N1G, N1B, N2G, N2B, LUB1, FFB1, FFB2) = range(20)
NPARAM = 20


def _ceil_div(a, b):
    return (a + b - 1) // b


def build_nc(N=2048, NT=512, use_bq=False, use_bk=False, use_bv=False,
             use_tmb=False, use_n1b=False, use_n2b=False, debug=False):
    NTILES = N // NT
    NCH = NT // P          # 128-token chunks per tile
    TOTCH = N // P
    W = NT + 4             # phase-C tile width with +-2 halo
    assert N % NT == 0 and NT % P == 0

    nc = bacc.Bacc(None, target_bir_lowering=False, debug=debug)

    xT_d = nc.dram_tensor("x_T", [D, N], BF16, kind="ExternalInput")
    w_d = {}
    for name in ("wqT", "wkT", "wvT", "w1T", "w2T", "f1T", "f2T"):
        w_d[name] = nc.dram_tensor(name, [D, D], BF16, kind="ExternalInput")
    pp_d = nc.dram_tensor("pp", [P, DO, NPARAM], F32, kind="ExternalInput")
    rows_d = nc.dram_tensor("rows", [1, 3 * D], BF16, kind="ExternalInput")
    yT_d = nc.dram_tensor("y_T", [D, N], F32, kind="ExternalOutput")

    qp_sp = nc.dram_tensor("qp_sp", [D, N], BF16)
    kp_sp = nc.dram_tensor("kp_sp", [N, D], BF16)
    v_sp = nc.dram_tensor("v_sp", [N, D], BF16)
    acc_sp = nc.dram_tensor("acc_sp", [D, N], F32)

    xT = xT_d.rearrange("(o p) n -> p o n", p=P)
    wr = {k: v.rearrange("(o p) n -> p o n", p=P) for k, v in w_d.items()}
    qp_r = qp_sp.rearrange("(o p) n -> p o n", p=P)
    kp_r = kp_sp.rearrange("(c p) d -> p c d", p=P)
    v_r = v_sp.rearrange("(c p) d -> p c d", p=P)
    acc_r = acc_sp.rearrange("(o p) n -> p o n", p=P)
    yT = yT_d.rearrange("(o p) n -> p o n", p=P)

    with tile.TileContext(nc) as tc, ExitStack() as top:
        persist = top.enter_context(tc.tile_pool(name="persist", bufs=1))
        pp = persist.tile([P, DO, NPARAM], F32)
        nc.sync.dma_start(pp, pp_d[:])
        rows = persist.tile([1, 3 * D], BF16)
        nc.sync.dma_start(rows, rows_d[:])
        ones_row = persist.tile([1, NT], BF16)
        nc.vector.memset(ones_row, 1.0)
        ones_1p_bf = persist.tile([1, P], BF16)
        nc.vector.memset(ones_1p_bf, 1.0)
        ones_1p_f32 = persist.tile([1, P], F32)
        nc.vector.memset(ones_1p_f32, 1.0)
        ones_col = persist.tile([P, 1], BF16)
        nc.vector.memset(ones_col, 1.0)
        ones_one = persist.tile([1, 1], BF16)
        nc.vector.memset(ones_one, 1.0)
        ksrow_sb = persist.tile([1, D], BF16)
        onesD_bf = persist.tile([P, P], BF16)
        nc.vector.memset(onesD_bf, 1.0 / D)
        onesD_f32 = persist.tile([P, P], F32)
        nc.vector.memset(onesD_f32, 1.0 / D)
        eps_ln = persist.tile([P, 1], F32)
        nc.vector.memset(eps_ln, 1e-5)
        kv_sb = persist.tile([P, DO, D], BF16)
        ksum_sb = persist.tile([P, DO, 1], BF16)
        rr_sb = persist.tile([1, N], F32)

        def stats_mm(psum, lhs_ones, rhs3, width):
            """Accumulate over DO k-chunks: psum[:, j] = mean over channel dim,
            replicated across partitions.  rhs3: [P, DO, width]."""
            for c0 in range(0, width, 512):
                cw = min(512, width - c0)
                for kc in range(DO):
                    nc.tensor.matmul(psum[:, c0:c0 + cw], lhs_ones,
                                     rhs3[:, kc, c0:c0 + cw],
                                     start=(kc == 0), stop=(kc == DO - 1))

        # ---------------- Phase A: QKV ----------------
        with ExitStack() as ph:
            wpool = ph.enter_context(tc.tile_pool(name="wA", bufs=1))
            wq_sb = wpool.tile([P, DO, D], BF16, tag="wq")
            nc.sync.dma_start(wq_sb, wr["wqT"])
            wk_sb = wpool.tile([P, DO, D], BF16, tag="wk")
            nc.sync.dma_start(wk_sb, wr["wkT"])
            wv_sb = wpool.tile([P, DO, D], BF16, tag="wv")
            nc.sync.dma_start(wv_sb, wr["wvT"])
            io = ph.enter_context(tc.tile_pool(name="ioA", bufs=2))
            ev = ph.enter_context(tc.tile_pool(name="evA", bufs=3))
            ps = ph.enter_context(tc.tile_pool(name="psA", bufs=2, space="PSUM"))
            ksp = ph.enter_context(tc.tile_pool(name="ksA", bufs=1, space="PSUM"))
            ps_ks = ksp.tile([1, D], F32, tag="ksrow")

            for it in range(NTILES):
                n0 = it * NT
                x_t = io.tile([P, DO, NT], BF16, tag="xA")
                nc.sync.dma_start(x_t, xT[:, :, n0:n0 + NT])

                # ---- Q: layout B, out [dout-chunk, n] ----
                qp_t = io.tile([P, DO, NT], BF16, tag="qpA")
                for dc in range(DO):
                    ps_q = ps.tile([P, NT], F32, tag="psq")
                    for kc in range(DO):
                        nc.tensor.matmul(ps_q, wq_sb[:, kc, dc * P:(dc + 1) * P],
                                         x_t[:, kc, :], start=(kc == 0),
                                         stop=(kc == DO - 1 and not use_bq))
                    if use_bq:
                        nc.tensor.matmul(ps_q, rows[0:1, dc * P:(dc + 1) * P],
                                         ones_row[0:1, :], start=False, stop=True)
                    m_t = ev.tile([P, NT], BF16, tag="mA")
                    nc.vector.tensor_scalar_min(m_t, ps_q, 0.0)
                    e_t = ev.tile([P, NT], BF16, tag="eA")
                    nc.scalar.activation(e_t, m_t, AF.Exp)
                    # elu(q)+1 = max(q,0) + exp(min(q,0))
                    nc.vector.scalar_tensor_tensor(qp_t[:, dc, :], ps_q, 0.0, e_t,
                                                   OP.max, OP.add)
                nc.sync.dma_start(qp_r[:, :, n0:n0 + NT], qp_t)

                # ---- K, V: layout A, out [token-chunk, dout] ----
                kp_t = io.tile([P, NCH, D], BF16, tag="kpA")
                v_t = io.tile([P, NCH, D], BF16, tag="vA")
                for ch in range(NCH):
                    cs = slice(ch * P, (ch + 1) * P)
                    for half in range(D // 512):
                        hs = slice(half * 512, (half + 1) * 512)
                        ps_k = ps.tile([P, 512], F32, tag="pskv")
                        for kc in range(DO):
                            nc.tensor.matmul(ps_k, x_t[:, kc, cs], wk_sb[:, kc, hs],
                                             start=(kc == 0),
                                             stop=(kc == DO - 1 and not use_bk))
                        if use_bk:
                            nc.tensor.matmul(ps_k, ones_1p_bf[0:1, :],
                                             rows[0:1, D + half * 512:D + (half + 1) * 512],
                                             start=False, stop=True)
                        m2 = ev.tile([P, 512], BF16, tag="mA2")
                        nc.vector.tensor_scalar_min(m2, ps_k, 0.0)
                        e2 = ev.tile([P, 512], BF16, tag="eA2")
                        nc.scalar.activation(e2, m2, AF.Exp)
                        nc.vector.scalar_tensor_tensor(kp_t[:, ch, hs], ps_k, 0.0,
                                                       e2, OP.max, OP.add)

                        ps_v = ps.tile([P, 512], F32, tag="pskv")
                        for kc in range(DO):
                            nc.tensor.matmul(ps_v, x_t[:, kc, cs], wv_sb[:, kc, hs],
                                             start=(kc == 0),
                                             stop=(kc == DO - 1 and not use_bv))
                        if use_bv:
                            nc.tensor.matmul(ps_v, ones_1p_bf[0:1, :],
                                             rows[0:1, 2 * D + half * 512:2 * D + (half + 1) * 512],
                                             start=False, stop=True)
                        nc.scalar.activation(v_t[:, ch, hs], ps_v, AF.Copy)
                    # K_sum accumulation as a [1, D] row (tokens on partitions)
                    for half in range(D // 512):
                        hs = slice(half * 512, (half + 1) * 512)
                        nc.tensor.matmul(ps_ks[0:1, hs], ones_col, kp_t[:, ch, hs],
                                         start=(it == 0 and ch == 0),
                                         stop=(it == NTILES - 1 and ch == NCH - 1))
                nc.sync.dma_start(kp_r[:, it * NCH:(it + 1) * NCH, :], kp_t)
                nc.sync.dma_start(v_r[:, it * NCH:(it + 1) * NCH, :], v_t)
            nc.scalar.activation(ksrow_sb, ps_ks[0:1, :], AF.Copy)
            # transpose K_sum row -> per-partition column layout [P, DO]
            ps_ksc = ksp.tile([P, DO], F32, tag="kscol")
            for dc in range(DO):
                nc.tensor.matmul(ps_ksc[:, dc:dc + 1],
                                 ksrow_sb[0:1, dc * P:(dc + 1) * P],
                                 ones_one[0:1, 0:1], start=True, stop=True)
            nc.scalar.activation(ksum_sb[:, :, 0], ps_ksc, AF.Copy)

        # ---------------- Phase B1: attention norm rows ----------------
        with ExitStack() as ph:
            io = ph.enter_context(tc.tile_pool(name="ioB1", bufs=2))
            sm1 = ph.enter_context(tc.tile_pool(name="smB1", bufs=2))
            ps1 = ph.enter_context(tc.tile_pool(name="psB1", bufs=2, space="PSUM"))
            for it in range(NTILES):
                n0 = it * NT
                qp_n = io.tile([P, DO, NT], BF16, tag="qpB1")
                nc.sync.dma_start(qp_n, qp_r[:, :, n0:n0 + NT])
                ps_n = ps1.tile([1, NT], F32, tag="psnB")
                for kc in range(DO):
                    nc.tensor.matmul(ps_n[0:1, :], ksum_sb[:, kc, :], qp_n[:, kc, :],
                                     start=(kc == 0), stop=(kc == DO - 1))
                nr = sm1.tile([1, NT], F32, tag="nrB")
                nc.vector.tensor_scalar_add(nr, ps_n[0:1, :], 1e-6)
                nc.vector.reciprocal_approx_fast(out=rr_sb[0:1, n0:n0 + NT], in_=nr)

        # ---------------- Phase B: KV accumulation ----------------
        with ExitStack() as ph:
            io = ph.enter_context(tc.tile_pool(name="ioB", bufs=4))
            ps = ph.enter_context(tc.tile_pool(name="psB", bufs=1, space="PSUM"))
            for eh in range(D // 512):
                hs = slice(eh * 512, (eh + 1) * 512)
                kv_ps = [ps.tile([P, 512], F32, tag=f"kvps{d}", name=f"kvps{d}")
                         for d in range(DO)]
                for ch in range(TOTCH):
                    kp_c = io.tile([P, D], BF16, tag="kpB")
                    nc.sync.dma_start(kp_c, kp_r[:, ch, :])
                    v_c = io.tile([P, 512], BF16, tag="vB")
                    nc.sync.dma_start(v_c, v_r[:, ch, hs])
                    for dc in range(DO):
                        nc.tensor.matmul(kv_ps[dc], kp_c[:, dc * P:(dc + 1) * P],
                                         v_c, start=(ch == 0), stop=(ch == TOTCH - 1))
                for dc in range(DO):
                    nc.scalar.activation(kv_sb[:, dc, hs], kv_ps[dc], AF.Copy)

        # ---------------- Phases C+D (shared FFN-weight prefetch) ----------------
        cd = top.enter_context(ExitStack())
        wpoolD = cd.enter_context(tc.tile_pool(name="wDpre", bufs=1))
        f1_sb = wpoolD.tile([P, DO, D], BF16, tag="f1")
        nc.sync.dma_start(f1_sb, wr["f1T"])
        f2_sb = wpoolD.tile([P, DO, D], BF16, tag="f2")
        nc.sync.dma_start(f2_sb, wr["f2T"])

        # ---------------- Phase C: conv'' + local MLP + token mixer ----------------
        with ExitStack() as ph:
            wpool = ph.enter_context(tc.tile_pool(name="wC", bufs=1))
            w1_sb = wpool.tile([P, DO, D], BF16, tag="w1")
            nc.sync.dma_start(w1_sb, wr["w1T"])
            w2_sb = wpool.tile([P, DO, D], BF16, tag="w2")
            nc.sync.dma_start(w2_sb, wr["w2T"])
            io = ph.enter_context(tc.tile_pool(name="ioC", bufs=2))
            mid = ph.enter_context(tc.tile_pool(name="midC", bufs=1))
            sm = ph.enter_context(tc.tile_pool(name="smC", bufs=1))
            ps = ph.enter_context(tc.tile_pool(name="psC", bufs=2, space="PSUM"))
            pst = ph.enter_context(tc.tile_pool(name="pstC", bufs=1, space="PSUM"))

            for it in range(NTILES):
                n0 = it * NT
                x_t = io.tile([P, DO, W], BF16, tag="xC")
                lo, hi = n0 - 2, n0 + NT + 2
                if lo < 0:
                    nc.vector.memset(x_t[:, :, 0:2], 0.0)
                    nc.sync.dma_start(x_t[:, :, 2:W], xT[:, :, 0:hi])
                elif hi > N:
                    nc.vector.memset(x_t[:, :, W - 2:W], 0.0)
                    nc.sync.dma_start(x_t[:, :, 0:W - 2], xT[:, :, lo:N])
                else:
                    nc.sync.dma_start(x_t, xT[:, :, lo:hi])

                acc = io.tile([P, DO, NT], F32, tag="accC")
                # diffusion dwconv'': center tap on ACT, side taps on DVE
                for o in range(DO):
                    nc.scalar.activation(acc[:, o, :], x_t[:, o, 2:NT + 2],
                                         AF.Identity, bias=pp[:, o, CB:CB + 1],
                                         scale=pp[:, o, C1:C1 + 1])
                for o in range(DO):
                    nc.vector.scalar_tensor_tensor(acc[:, o, :], x_t[:, o, 1:NT + 1],
                                                   pp[:, o, C0:C0 + 1], acc[:, o, :],
                                                   OP.mult, OP.add)
                for o in range(DO):
                    nc.vector.scalar_tensor_tensor(acc[:, o, :], x_t[:, o, 3:NT + 3],
                                                   pp[:, o, C2:C2 + 1], acc[:, o, :],
                                                   OP.mult, OP.add)

                # local MLP
                h1_t = mid.tile([P, DO, NT], BF16, tag="h1")
                for dc in range(DO):
                    ps_h = ps.tile([P, NT], F32, tag="psh1")
                    for kc in range(DO):
                        nc.tensor.matmul(ps_h, w1_sb[:, kc, dc * P:(dc + 1) * P],
                                         x_t[:, kc, 2:NT + 2],
                                         start=(kc == 0), stop=(kc == DO - 1))
                    nc.scalar.activation(h1_t[:, dc, :], ps_h, AF.Gelu,
                                         bias=pp[:, dc, LUB1:LUB1 + 1])
                for dc in range(DO):
                    ps_h = ps.tile([P, NT], F32, tag="psh2")
                    for kc in range(DO):
                        nc.tensor.matmul(ps_h, w2_sb[:, kc, dc * P:(dc + 1) * P],
                                         h1_t[:, kc, :],
                                         start=(kc == 0), stop=(kc == DO - 1))
                    nc.vector.tensor_add(acc[:, dc, :], acc[:, dc, :], ps_h)

                # token mixer: LN over channels (stats via ones-matmul)
                sq_t = mid.tile([P, DO, W], BF16, tag="tokA")
                nc.scalar.activation(sq_t, x_t, AF.Square)
                ps_m = pst.tile([P, W], F32, tag="psm")
                stats_mm(ps_m, onesD_bf, x_t, W)
                ps_s = pst.tile([P, W], F32, tag="pss")
                stats_mm(ps_s, onesD_bf, sq_t, W)
                m_sb = sm.tile([P, W], BF16, tag="msb")
                nc.scalar.activation(m_sb, ps_m, AF.Copy)
                var = sm.tile([P, W], F32, tag="var")
                nc.scalar.activation(var, ps_m, AF.Square)
                nc.vector.tensor_sub(var, ps_s, var)
                std = sm.tile([P, W], F32, tag="std")
                nc.scalar.activation(std, var, AF.Sqrt, bias=eps_ln[:, 0:1])
                rstd_f = sm.tile([P, W], F32, tag="rstdf")
                nc.vector.reciprocal_approx_fast(out=rstd_f, in_=std)
                rstd = sm.tile([P, W], BF16, tag="rstd")
                nc.vector.tensor_copy(rstd, rstd_f)
                u_t = mid.tile([P, DO, W], BF16, tag="tokB")
                for o in range(DO):
                    nc.vector.tensor_sub(u_t[:, o, :], x_t[:, o, :], m_sb)
                xm_t = mid.tile([P, DO, W], BF16, tag="tokC")
                for o in range(DO):
                    nc.vector.scalar_tensor_tensor(xm_t[:, o, :], u_t[:, o, :],
                                                   pp[:, o, TMG:TMG + 1], rstd,
                                                   OP.mult, OP.mult)
                if use_tmb:
                    for o in range(DO):
                        nc.vector.tensor_scalar_add(xm_t[:, o, :], xm_t[:, o, :],
                                                    pp[:, o, TMB:TMB + 1])
                # conv1: t_s[k] = conv1(xm)[k+1], k in [0, W-2) (shifted for alignment)
                t_t = mid.tile([P, DO, W - 2], BF16, tag="tokA")
                for o in range(DO):
                    nc.scalar.activation(t_t[:, o, :], xm_t[:, o, 1:W - 1],
                                         AF.Identity, bias=pp[:, o, TCB1:TCB1 + 1],
                                         scale=pp[:, o, T1:T1 + 1])
                for o in range(DO):
                    nc.vector.scalar_tensor_tensor(t_t[:, o, :], xm_t[:, o, 0:W - 2],
                                                   pp[:, o, T0:T0 + 1],
                                                   t_t[:, o, :], OP.mult, OP.add)
                for o in range(DO):
                    nc.vector.scalar_tensor_tensor(t_t[:, o, :], xm_t[:, o, 2:W],
                                                   pp[:, o, T2:T2 + 1],
                                                   t_t[:, o, :], OP.mult, OP.add)
                t2_t = mid.tile([P, DO, W - 2], BF16, tag="tokB")
                nc.scalar.activation(t2_t, t_t, AF.Gelu)
                # conv2's zero padding applies to gelu(conv1) outside the sequence
                if it == 0:
                    nc.vector.memset(t2_t[:, :, 0:1], 0.0)
                if it == NTILES - 1:
                    nc.vector.memset(t2_t[:, :, W - 3:W - 2], 0.0)
                # conv2 valid on the NT output cols; t2_s[k] holds token n0-1+k
                t3_t = mid.tile([P, DO, NT], BF16, tag="tokC")
                for o in range(DO):
                    nc.scalar.activation(t3_t[:, o, :], t2_t[:, o, 1:NT + 1],
                                         AF.Identity, bias=0.0,
                                         scale=pp[:, o, U1:U1 + 1])
                for o in range(DO):
                    nc.vector.scalar_tensor_tensor(t3_t[:, o, :], t2_t[:, o, 0:NT],
                                                   pp[:, o, U0:U0 + 1], t3_t[:, o, :],
                                                   OP.mult, OP.add)
                for o in range(DO):
                    nc.vector.scalar_tensor_tensor(t3_t[:, o, :], t2_t[:, o, 2:NT + 2],
                                                   pp[:, o, U2:U2 + 1], t3_t[:, o, :],
                                                   OP.mult, OP.add)
                for o in range(DO):
                    nc.vector.tensor_add(acc[:, o, :], acc[:, o, :], t3_t[:, o, :])
                nc.sync.dma_start(acc_r[:, :, n0:n0 + NT], acc)

        # ---------------- Phase D: attention + LN1 + FFN + LN2 ----------------
        with ExitStack() as ph:
            io = ph.enter_context(tc.tile_pool(name="ioD", bufs=2))
            mid = ph.enter_context(tc.tile_pool(name="midD", bufs=1))
            sm = ph.enter_context(tc.tile_pool(name="smD", bufs=1))
            ps = ph.enter_context(tc.tile_pool(name="psD", bufs=2, space="PSUM"))
            pst = ph.enter_context(tc.tile_pool(name="pstD", bufs=1, space="PSUM"))

            for it in range(NTILES):
                n0 = it * NT
                qp_t = io.tile([P, DO, NT], BF16, tag="qpD")
                nc.sync.dma_start(qp_t, qp_r[:, :, n0:n0 + NT])
                acc_t = io.tile([P, DO, NT], F32, tag="accD")
                nc.sync.dma_start(acc_t, acc_r[:, :, n0:n0 + NT])

                # replicate 1/norm across partitions, fold into Qp
                ps_rep = pst.tile([P, NT], F32, tag="psrep")
                nc.tensor.matmul(ps_rep, ones_1p_f32[0:1, :],
                                 rr_sb[0:1, n0:n0 + NT], start=True, stop=True)
                rep_sb = mid.tile([P, NT], BF16, tag="repsb")
                nc.scalar.activation(rep_sb, ps_rep, AF.Copy)
                for kc in range(DO):
                    nc.vector.tensor_mul(qp_t[:, kc, :], qp_t[:, kc, :], rep_sb)

                # numerator (with 1/norm folded into Qp'), accumulate into acc
                for ec in range(DO):
                    ps_u = ps.tile([P, NT], F32, tag="psnum")
                    for kc in range(DO):
                        nc.tensor.matmul(ps_u, kv_sb[:, kc, ec * P:(ec + 1) * P],
                                         qp_t[:, kc, :],
                                         start=(kc == 0), stop=(kc == DO - 1))
                    nc.vector.tensor_add(acc_t[:, ec, :], acc_t[:, ec, :], ps_u)

                # LN1
                sq_t = mid.tile([P, DO, NT], BF16, tag="sqD")
                nc.scalar.activation(sq_t, acc_t, AF.Square)
                ps_m1 = pst.tile([P, NT], F32, tag="psm1")
                stats_mm(ps_m1, onesD_f32, acc_t, NT)
                ps_s1 = pst.tile([P, NT], F32, tag="pss1")
                stats_mm(ps_s1, onesD_bf, sq_t, NT)
                m1_sb = sm.tile([P, NT], F32, tag="m1sb")
                nc.scalar.activation(m1_sb, ps_m1, AF.Copy)
                var1 = sm.tile([P, NT], F32, tag="varD")
                nc.scalar.activation(var1, ps_m1, AF.Square)
                nc.vector.tensor_sub(var1, ps_s1, var1)
                std1 = sm.tile([P, NT], F32, tag="stdD")
                nc.scalar.activation(std1, var1, AF.Sqrt, bias=eps_ln[:, 0:1])
                rstd1_f = sm.tile([P, NT], F32, tag="rstdf")
                nc.vector.reciprocal_approx_fast(out=rstd1_f, in_=std1)
                rstd1 = sm.tile([P, NT], BF16, tag="rstdb")
                nc.vector.tensor_copy(rstd1, rstd1_f)
                u1_t = mid.tile([P, DO, NT], BF16, tag="u1")
                for o in range(DO):
                    nc.vector.tensor_sub(u1_t[:, o, :], acc_t[:, o, :], m1_sb)
                y1_t = mid.tile([P, DO, NT], BF16, tag="y1")
                for o in range(DO):
                    nc.vector.scalar_tensor_tensor(y1_t[:, o, :], u1_t[:, o, :],
                                                   pp[:, o, N1G:N1G + 1], rstd1,
                                                   OP.mult, OP.mult)
                if use_n1b:
                    for o in range(DO):
                        nc.vector.tensor_scalar_add(y1_t[:, o, :], y1_t[:, o, :],
                                                    pp[:, o, N1B:N1B + 1])

                # FFN
                f1h_t = mid.tile([P, DO, NT], BF16, tag="f1h")
                for dc in range(DO):
                    ps_f = ps.tile([P, NT], F32, tag="psf")
                    for kc in range(DO):
                        nc.tensor.matmul(ps_f, f1_sb[:, kc, dc * P:(dc + 1) * P],
                                         y1_t[:, kc, :],
                                         start=(kc == 0), stop=(kc == DO - 1))
                    nc.scalar.activation(f1h_t[:, dc, :], ps_f, AF.Gelu,
                                         bias=pp[:, dc, FFB1:FFB1 + 1])
                y2_t = mid.tile([P, DO, NT], F32, tag="y2")
                for dc in range(DO):
                    ps_f = ps.tile([P, NT], F32, tag="psf")
                    for kc in range(DO):
                        nc.tensor.matmul(ps_f, f2_sb[:, kc, dc * P:(dc + 1) * P],
                                         f1h_t[:, kc, :],
                                         start=(kc == 0), stop=(kc == DO - 1))
                    # y2 = (f2 + ff_b2) + y1
                    nc.vector.scalar_tensor_tensor(y2_t[:, dc, :], ps_f,
                                                   pp[:, dc, FFB2:FFB2 + 1],
                                                   y1_t[:, dc, :], OP.add, OP.add)

                # LN2 -> output
                sq2_t = mid.tile([P, DO, NT], BF16, tag="sqD")
                nc.scalar.activation(sq2_t, y2_t, AF.Square)
                ps_m2 = pst.tile([P, NT], F32, tag="psm1")
                stats_mm(ps_m2, onesD_f32, y2_t, NT)
                ps_s2 = pst.tile([P, NT], F32, tag="pss1")
                stats_mm(ps_s2, onesD_bf, sq2_t, NT)
                m2_sb = sm.tile([P, NT], F32, tag="m1sb")
                nc.scalar.activation(m2_sb, ps_m2, AF.Copy)
                var2 = sm.tile([P, NT], F32, tag="varD")
                nc.scalar.activation(var2, ps_m2, AF.Square)
                nc.vector.tensor_sub(var2, ps_s2, var2)
                std2 = sm.tile([P, NT], F32, tag="stdD")
                nc.scalar.activation(std2, var2, AF.Sqrt, bias=eps_ln[:, 0:1])
                rstd2 = sm.tile([P, NT], F32, tag="rstdf")
                nc.vector.reciprocal_approx_fast(out=rstd2, in_=std2)
                yo_t = io.tile([P, DO, NT], F32, tag="yo")
                for o in range(DO):
                    nc.vector.tensor_sub(yo_t[:, o, :], y2_t[:, o, :], m2_sb)
                for o in range(DO):
                    nc.vector.scalar_tensor_tensor(yo_t[:, o, :], yo_t[:, o, :],
                                                   pp[:, o, N2G:N2G + 1], rstd2,
                                                   OP.mult, OP.mult)
                if use_n2b:
                    for o in range(DO):
                        nc.vector.tensor_scalar_add(yo_t[:, o, :], yo_t[:, o, :],
                                                    pp[:, o, N2B:N2B + 1])
                nc.sync.dma_start(yT[:, :, n0:n0 + NT], yo_t)

    nc.compile()
    return nc


def make_in_maps(inputs, n_cores=8):
    """Host-side preprocessing: fold constants, transpose, cast, shard."""
    x = np.asarray(inputs["x"], np.float32)
    B, N, D_ = x.shape
    dt = float(np.asarray(inputs["delta_t"]))

    def g(k):
        return np.asarray(inputs[k], np.float32)

    diff_w, diff_b = g("diff_w"), g("diff_b")
    tm_w1, tm_cb1 = g("tm_w1"), g("tm_cb1")
    tm_w2, tm_cb2 = g("tm_w2"), g("tm_cb2")

    pp = np.zeros((P, DO, NPARAM), np.float32)

    def put(i, v):
        pp[:, :, i] = v.reshape(DO, P).T

    put(C0, dt * diff_w[:, 0, 0])
    put(C1, dt * diff_w[:, 0, 1] + (1.0 - dt))
    put(C2, dt * diff_w[:, 0, 2])
    put(CB, dt * diff_b + g("lu_b2") + tm_cb2)
    put(T0, tm_w1[:, 0, 0])
    put(T1, tm_w1[:, 0, 1])
    put(T2, tm_w1[:, 0, 2])
    put(TCB1, tm_cb1)
    put(U0, tm_w2[:, 0, 0])
    put(U1, tm_w2[:, 0, 1])
    put(U2, tm_w2[:, 0, 2])
    put(TMG, g("tm_g"))
    put(TMB, g("tm_beta"))
    put(N1G, g("n1_g"))
    put(N1B, g("n1_b"))
    put(N2G, g("n2_g"))
    put(N2B, g("n2_b"))
    put(LUB1, g("lu_b1"))
    put(FFB1, g("ff_b1"))
    put(FFB2, g("ff_b2"))

    rows = np.zeros((1, 3 * D), np.float32)
    rows[0, 0:D] = g("bq")
    rows[0, D:2 * D] = g("bk")
    rows[0, 2 * D:3 * D] = g("bv")
    rows = rows.astype(BF16_NP)

    wt = {}
    for name, key in (("wqT", "wq"), ("wkT", "wk"), ("wvT", "wv"),
                      ("w1T", "lu_w1"), ("w2T", "lu_w2"),
                      ("f1T", "ff_w1"), ("f2T", "ff_w2")):
        wt[name] = np.ascontiguousarray(g(key).T).astype(BF16_NP)

    xT = np.ascontiguousarray(x.transpose(0, 2, 1)).astype(BF16_NP)

    flags = dict(
        use_bq=bool(np.any(g("bq"))),
        use_bk=bool(np.any(g("bk"))),
        use_bv=bool(np.any(g("bv"))),
        use_tmb=bool(np.any(g("tm_beta"))),
        use_n1b=bool(np.any(g("n1_b"))),
        use_n2b=bool(np.any(g("n2_b"))),
    )

    shared = {**wt, "pp": pp, "rows": rows}
    in_maps = [{**shared, "x_T": xT[b]} for b in range(n_cores)]
    return in_maps, flags, (B, N)


_NC_CACHE = {}


def kernel(**inputs):
    in_maps, flags, (B, N) = make_in_maps(inputs)
    key = (N, tuple(sorted(flags.items())))
    if key not in _NC_CACHE:
        _NC_CACHE[key] = build_nc(N=N, NT=512, **flags)
    nc = _NC_CACHE[key]
    res = run_bass_kernel_spmd(nc, in_maps, list(range(B)))
    y = np.stack([res.results[b]["y_T"] for b in range(B)])
    return np.ascontiguousarray(y.transpose(0, 2, 1)).astype(np.float32)
